# revision 3
# baseline (speedup 1.0000x reference)
"""LightGCN (3-layer) on 8 Trainium2 NeuronCores via Bass/Tile — v2.

Formulation (source-shard + ReduceScatter; no AllGather):
  Host precomputes norm_e = dinv[row]*w*dinv[col] (fp16), so the device only
  runs 3 SpMM layers: h_{l+1} = A h_l, out = alpha*(x + h1 + h2 + h3).
  Edges are sharded by SOURCE row: core c owns rows [c*12544, (c+1)*12544).
  Each layer: core gathers its LOCAL h rows per edge (dma_gather from a
  [12544, 128] fp16 duplicated-row table; idx fits int16), forms weighted
  messages rws = h[row]*norm via one broadcast-AP DVE mult per chunk, builds
  unweighted one-hot lhsT (iota==col) per chunk, and matmul-accumulates
  per-destination-block partial sums for ALL 100352 dest nodes (fp16 to DRAM).
  A ReduceScatter(add) then hands each core the finished rows it owns —
  output is only 1.6MB/core vs 25.7MB AllGather replication in v1.
  hsum accumulates own rows across layers; out = alpha*hsum + alpha*x.

Numerics: fp16 throughout (h, one-hot, weights, RS); PSUM accumulates fp32.
Rel err vs fp32 reference ~1e-3, gate is 2e-2.
"""

import numpy as np

N_NODES = 100000
D = 64
N_CORES = 8
BPC = 98                       # 128-node blocks per core (dest AND source)
NLOC = BPC * 128               # 12544 nodes per core
NPAD = N_CORES * NLOC          # 100352 padded node count
NBLK = N_CORES * BPC           # 784 global dest blocks
ALPHA = 0.25
N_LAYERS = 3
CG = 24                        # dest blocks per chunk
PIECE = 8                      # subblocks per dma_gather (1024 idx HW limit)


def _preprocess(x, edge_attr, edge_index, edge_mask):
    keep = np.asarray(edge_mask).astype(bool)
    row = np.asarray(edge_index[0]).astype(np.int64)[keep]
    col = np.asarray(edge_index[1]).astype(np.int64)[keep]
    w = np.asarray(edge_attr).astype(np.float32)[keep]

    deg = np.bincount(col, weights=w, minlength=N_NODES)
    dinv = np.zeros(N_NODES)
    nz = deg > 0
    dinv[nz] = 1.0 / np.sqrt(deg[nz])
    wn = (dinv[row] * w * dinv[col]).astype(np.float32)

    core = row // NLOC
    g = col >> 7
    key = core * NBLK + g
    order = np.argsort(key.astype(np.int16), kind="stable")  # radix sort
    key_s = key[order]
    wn_s = wn[order]
    colloc_s = (col[order] & 127).astype(np.float16)
    idx_s = (row[order] - core[order] * NLOC).astype(np.int16)

    counts = np.bincount(key_s, minlength=N_CORES * NBLK)
    S = (-(-counts.reshape(N_CORES, NBLK) // 128)).max(axis=0)  # [NBLK]
    sb0 = np.zeros(NBLK, np.int64)
    np.cumsum(S[:-1], out=sb0[1:])
    SB_TOT = int(S.sum())
    NSLOT = SB_TOT * 128

    starts = np.zeros(N_CORES * NBLK + 1, np.int64)
    np.cumsum(counts, out=starts[1:])
    rank = np.arange(len(key_s)) - np.repeat(starts[:-1], counts)
    slot = sb0[key_s % NBLK] * 128 + rank
    flat = (key_s // NBLK) * NSLOT + slot

    colv = np.zeros(N_CORES * NSLOT, np.float16)
    wv = np.zeros(N_CORES * NSLOT, np.float16)
    idxv = np.zeros(N_CORES * NSLOT, np.int16)
    colv[flat] = colloc_s
    wv[flat] = wn_s.astype(np.float16)
    idxv[flat] = idx_s

    iota = np.tile(np.arange(128, dtype=np.float16)[None, :], (128, 1))
    xf = np.asarray(x, np.float32)
    ins = []
    for c in range(N_CORES):
        coltab = colv[c * NSLOT:(c + 1) * NSLOT].reshape(SB_TOT, 128).T
        wtab = wv[c * NSLOT:(c + 1) * NSLOT].reshape(SB_TOT, 128).T
        idxw = idxv[c * NSLOT:(c + 1) * NSLOT].reshape(SB_TOT * 8, 16).T
        lo = c * NLOC
        hi = min((c + 1) * NLOC, N_NODES)
        xz = np.zeros((NLOC, D), np.float32)
        xz[:hi - lo] = xf[lo:hi]
        xa = (ALPHA * xz).reshape(BPC, 128, D).transpose(1, 0, 2)\
            .reshape(128, BPC * D).astype(np.float16)
        ctab = np.ascontiguousarray(
            np.concatenate([iota, coltab, wtab, xa], axis=1))
        ins.append({"ctab": ctab, "idxb": np.ascontiguousarray(idxw)})
    return ins, S, sb0, SB_TOT


def _build(S, sb0, SB_TOT):
    import concourse.bacc as bacc
    import concourse.mybir as mybir
    import concourse.tile as tile

    f16 = mybir.dt.float16
    f32 = mybir.dt.float32
    i16 = mybir.dt.int16
    Alu = mybir.AluOpType

    O_COL = 128
    O_W = O_COL + SB_TOT
    O_XA = O_W + SB_TOT
    CW = O_XA + BPC * D

    nc = bacc.Bacc("TRN2", target_bir_lowering=False, debug=False,
                   num_devices=N_CORES, dynamic_dma_scratch_size=32768)

    ct_in = nc.dram_tensor("ctab", [128, CW], f16, kind="ExternalInput")
    idx_in = nc.dram_tensor("idxb", [16, 8 * SB_TOT], i16, kind="ExternalInput")
    out_ext = nc.dram_tensor("outloc", [NLOC, D], f16, kind="ExternalOutput")
    htab = nc.dram_tensor("htab", [NLOC, 128], f16)
    hpart = nc.dram_tensor("hpart", [NPAD, D], f16)
    hnew = nc.dram_tensor("hnew", [NLOC, D], f16)
    rg = [list(range(N_CORES))]

    chunks = []
    for g0 in range(0, NBLK, CG):
        g1 = min(g0 + CG, NBLK)
        chunks.append((g0, g1, int(sb0[g0]),
                       int(sb0[g1 - 1] + S[g1 - 1] - sb0[g0])))

    nidx_regs = {}

    with tile.TileContext(nc) as tc:
        def nidx_reg(v):
            if v not in nidx_regs:
                nidx_regs[v] = nc.gpsimd.to_reg(v)
            return nidx_regs[v]

        with (
            tc.tile_pool(name="const", bufs=1) as constp,
            tc.tile_pool(name="acc", bufs=1) as accp,
            tc.tile_pool(name="dst", bufs=3) as dstp,
            tc.tile_pool(name="oh", bufs=3) as ohp,
            tc.tile_pool(name="rws", bufs=3) as rwsp,
            tc.tile_pool(name="hp", bufs=3) as hpp,
            tc.tile_pool(name="ps", bufs=2, space="PSUM") as psp,
            tc.tile_pool(name="mis", bufs=1) as misp,
        ):
            ct = constp.tile([128, CW], f16)
            nc.sync.dma_start(ct[:], ct_in.ap())
            idxt = constp.tile([128, 8 * SB_TOT], i16)
            for k in range(8):
                nc.sync.dma_start(idxt[16 * k:16 * (k + 1), :], idx_in.ap())
            iota = ct[:, 0:128]
            xa3 = ct[:, O_XA:O_XA + BPC * D].rearrange("p (g d) -> p g d", g=BPC)

            hsum = accp.tile([128, BPC, D], f32)

            # h0 table: htab = dup(x) = dup(4*xa)
            h0 = misp.tile([128, BPC, 128], f16, tag="hd")
            nc.vector.tensor_scalar(h0[:, :, 0:D], xa3, 4.0, None, op0=Alu.mult)
            nc.vector.tensor_scalar(h0[:, :, D:128], xa3, 4.0, None, op0=Alu.mult)
            nc.sync.dma_start(
                htab.ap().rearrange("(g p) d -> p g d", p=128), h0[:])

            for l in range(N_LAYERS):
                for ci, (g0, g1, base, ns) in enumerate(chunks):
                    if ns > 0:
                        dstt = dstp.tile([128, ns, 128], f16, tag="dst")
                        for p0 in range(0, ns, PIECE):
                            pe = min(p0 + PIECE, ns)
                            nv = (pe - p0) * 128
                            nc.gpsimd.dma_gather(
                                dstt[:, p0:pe, :], htab.ap(),
                                idxt[:, 8 * (base + p0):8 * (base + pe)],
                                nv, nidx_reg(nv), 128, queue_num=0)
                        oht = ohp.tile([128, ns, 128], f16, tag="oh")
                        nc.vector.tensor_tensor(
                            oht[:],
                            iota.unsqueeze(1).broadcast_to([128, ns, 128]),
                            ct[:, O_COL + base:O_COL + base + ns]
                            .unsqueeze(2).broadcast_to([128, ns, 128]),
                            op=Alu.is_equal)
                        rwst = rwsp.tile([128, ns, D], f16, tag="rws")
                        nc.vector.tensor_tensor(
                            rwst[:], dstt[:, :, 0:D],
                            ct[:, O_W + base:O_W + base + ns]
                            .unsqueeze(2).broadcast_to([128, ns, D]),
                            op=Alu.mult)
                    hp = hpp.tile([128, g1 - g0, D], f16, tag="hp")
                    ps = psp.tile([128, g1 - g0, D], f32, tag="ps")
                    for gg in range(g0, g1):
                        j = gg - g0
                        if S[gg] == 0:
                            nc.vector.memset(ps[:, j, :], 0.0)
                            continue
                        for s in range(S[gg]):
                            pos = int(sb0[gg]) - base + s
                            nc.tensor.matmul(
                                ps[:, j, :], oht[:, pos, :], rwst[:, pos, :],
                                start=(s == 0), stop=(s == S[gg] - 1))
                    nc.scalar.copy(hp[:], ps[:])
                    nc.sync.dma_start(
                        hpart.ap()[g0 * 128:g1 * 128, :]
                        .rearrange("(g p) d -> p g d", p=128), hp[:])
                nc.gpsimd.collective_compute(
                    "ReduceScatter", Alu.add, replica_groups=rg,
                    ins=[hpart.ap().opt()], outs=[hnew.ap().opt()])
                hn = misp.tile([128, BPC, D], f16, tag="hn")
                nc.sync.dma_start(
                    hn[:], hnew.ap().rearrange("(g p) d -> p g d", p=128))
                if l == 0:
                    nc.vector.tensor_copy(hsum[:], hn[:])
                else:
                    nc.vector.tensor_tensor(hsum[:], hsum[:], hn[:], op=Alu.add)
                if l < N_LAYERS - 1:
                    hd = misp.tile([128, BPC, 128], f16, tag="hd")
                    nc.scalar.copy(hd[:, :, 0:D], hn[:])
                    nc.scalar.copy(hd[:, :, D:128], hn[:])
                    nc.sync.dma_start(
                        htab.ap().rearrange("(g p) d -> p g d", p=128), hd[:])

            outt = misp.tile([128, BPC, D], f16, tag="out")
            nc.vector.scalar_tensor_tensor(
                outt[:], hsum[:], ALPHA, xa3, op0=Alu.mult, op1=Alu.add)
            nc.sync.dma_start(
                out_ext.ap().rearrange("(g p) d -> p g d", p=128), outt[:])
    nc.compile()
    return nc


def _make_runner(nc):
    """Return fn(ins_list) -> list of per-core output dicts, with the jitted
    executable cached across calls (the stock run_bass_kernel_spmd rebuilds
    the jit closure, costing ~2.6s/call under axon)."""
    from concourse._compat import axon_active
    from concourse import mybir as _mybir

    if not axon_active():
        # Native /dev/neuron* path: compile the NEFF once, then run it
        # directly per call (run_bass_kernel_spmd re-runs the BIR->NEFF
        # compile subprocess on every invocation, ~1.8s).
        import tempfile
        from concourse import bass_utils

        out_specs = []
        for alloc in nc.m.functions[0].allocations:
            if (isinstance(alloc, _mybir.MemoryLocationSet)
                    and alloc.kind == "ExternalOutput"):
                out_specs.append((alloc.memorylocations[0].name,
                                  tuple(alloc.tensor_shape),
                                  _mybir.dt.np(alloc.dtype)))
        state = {}

        def run_native(ins):
            if "neff" not in state:
                state["neff"] = bass_utils.compile_bass_kernel(
                    nc, tempfile.mkdtemp())
            in_maps = [dict(m) for m in ins]
            if nc.partition_id_tensor:
                for c, m in enumerate(in_maps):
                    m[nc.partition_id_tensor.name] = np.array(
                        [[c]], dtype=np.uint32)
            out_maps = [
                {name: np.zeros(shape, dt) for name, shape, dt in out_specs}
                for _ in range(N_CORES)]
            try:
                return bass_utils.run_neff(
                    state["neff"], in_maps, out_maps,
                    core_ids=list(range(N_CORES)),
                    has_collectives=nc.has_collectives)
            except Exception:
                return bass_utils.run_bass_kernel_spmd(
                    nc, ins, core_ids=list(range(N_CORES))).results
        return run_native

    import jax
    from jax.sharding import Mesh, PartitionSpec
    from jax.experimental.shard_map import shard_map
    from concourse import mybir
    from concourse.bass2jax import (
        _bass_exec_p, install_neuronx_cc_hook, partition_id_tensor)

    install_neuronx_cc_hook()
    partition_name = (nc.partition_id_tensor.name
                      if nc.partition_id_tensor else None)
    in_names, out_names, out_avals, zero_shapes = [], [], [], []
    for alloc in nc.m.functions[0].allocations:
        if not isinstance(alloc, mybir.MemoryLocationSet):
            continue
        name = alloc.memorylocations[0].name
        if alloc.kind == "ExternalInput":
            if name != partition_name:
                in_names.append(name)
        elif alloc.kind == "ExternalOutput":
            shape = tuple(alloc.tensor_shape)
            dtype = mybir.dt.np(alloc.dtype)
            out_names.append(name)
            out_avals.append(jax.core.ShapedArray(shape, dtype))
            zero_shapes.append((shape, dtype))
    n_params = len(in_names)
    n_outs = len(out_names)
    in_names_all = in_names + out_names + (
        [partition_name] if partition_name else [])
    donate = tuple(range(n_params, n_params + n_outs))

    def _body(*args):
        operands = list(args)
        if partition_name is not None:
            operands.append(partition_id_tensor())
        return tuple(_bass_exec_p.bind(
            *operands, out_avals=tuple(out_avals),
            in_names=tuple(in_names_all), out_names=tuple(out_names),
            lowering_input_output_aliases=(), sim_require_finite=True,
            sim_require_nnan=True, nc=nc))

    devices = jax.devices()[:N_CORES]
    mesh = Mesh(np.asarray(devices), ("core",))
    sharded = jax.jit(
        shard_map(_body, mesh=mesh,
                  in_specs=(PartitionSpec("core"),) * (n_params + n_outs),
                  out_specs=(PartitionSpec("core"),) * n_outs,
                  check_rep=False),
        donate_argnums=donate, keep_unused=True)

    def run_axon(ins):
        concat_in = [
            np.concatenate([np.asarray(ins[c][name]) for c in range(N_CORES)],
                           axis=0)
            for name in in_names]
        concat_zeros = [
            np.zeros((N_CORES * s[0], *s[1:]), dt) for s, dt in zero_shapes]
        out_arrs = sharded(*concat_in, *concat_zeros)
        return [
            {name: np.asarray(out_arrs[i]).reshape(
                N_CORES, *zero_shapes[i][0])[c]
             for i, name in enumerate(out_names)}
            for c in range(N_CORES)]
    return run_axon


_CACHE = {}


def kernel(x, edge_attr, edge_index, edge_mask):
    ins, S, sb0, SB_TOT = _preprocess(x, edge_attr, edge_index, edge_mask)
    ck = (SB_TOT, S.tobytes())
    if ck not in _CACHE:
        nc = _build(S, sb0, SB_TOT)
        _CACHE[ck] = (nc, _make_runner(nc))
    nc, run = _CACHE[ck]
    results = run(ins)
    out = np.concatenate(
        [results[c]["outloc"] for c in range(N_CORES)], axis=0)[:N_NODES]
    return out.astype(np.float32)
